# revision 1
# baseline (speedup 1.0000x reference)
"""Trainium2 Bass kernel for nn_BLCD_Loss (retrieval_knn).

Math: for l2-normalized rows, ||a-b||^2 = 2 - 2*a.b, so all pairwise
distances come from two small Gram matmuls per core. The top-(K+1)
neighbor selection reduces to a per-row threshold (17th largest cosine)
found with two rounds of the DVE 8-wide `max` + `match_replace` ops, and
the neighbor gather becomes a 0/1 mask multiply. Self-pairs are excluded
up-front by subtracting a large constant on the (local) diagonal.

Sharding: 256 anchor rows -> 32 rows on each of 8 cores; each core gets
the full yi^T (256KB) plus its local slices, computes a partial scalar
loss; the host sums the 8 partials.
"""

import numpy as np

N, D, K = 256, 256, 16
M_MARGIN, T_THRESH, EPS = 0.6, 0.0025, 1e-12
NCORES, RPC = 8, 32  # cores, rows per core
BIG = 1000.0

_CACHE = {}


def _build():
    from concourse import bacc, mybir, tile
    import concourse.bass as bass

    dt = mybir.dt.float32
    Alu = mybir.AluOpType
    Act = mybir.ActivationFunctionType

    nc = bacc.Bacc("TRN2", target_bir_lowering=False, debug=False)

    yiT_d = nc.dram_tensor("yiT", [D, N], dt, kind="ExternalInput")
    yiLT_d = nc.dram_tensor("yiLT", [D, RPC], dt, kind="ExternalInput")
    yitT_d = nc.dram_tensor("yitT", [D, RPC], dt, kind="ExternalInput")
    ylcat_d = nc.dram_tensor("ylcat", [RPC, 2 * D], dt, kind="ExternalInput")
    eyeB_d = nc.dram_tensor("eyeB", [RPC, N], dt, kind="ExternalInput")
    i32_d = nc.dram_tensor("i32", [RPC, RPC], dt, kind="ExternalInput")
    eyeN_d = nc.dram_tensor("eyeN", [RPC, N], dt, kind="ExternalInput")
    out_d = nc.dram_tensor("out", [1, 1], dt, kind="ExternalOutput")

    with tile.TileContext(nc) as tc:
        with (
            tc.tile_pool(name="sb", bufs=1) as sb,
            tc.tile_pool(name="ps", bufs=1, space=bass.MemorySpace.PSUM) as ps,
        ):
            yiT0 = sb.tile([128, N], dt)
            yiT1 = sb.tile([128, N], dt)
            nc.sync.dma_start(yiT0[0:64, :], yiT_d[0:64, :])
            nc.sync.dma_start(yiT0[64:128, :], yiT_d[64:128, :])
            nc.sync.dma_start(yiT1[0:64, :], yiT_d[128:192, :])
            nc.sync.dma_start(yiT1[64:128, :], yiT_d[192:256, :])
            yiLT0 = sb.tile([128, RPC], dt)
            yiLT1 = sb.tile([128, RPC], dt)
            nc.sync.dma_start(yiLT0[:], yiLT_d[0:128, :])
            nc.sync.dma_start(yiLT1[:], yiLT_d[128:256, :])
            yitT0 = sb.tile([128, RPC], dt)
            yitT1 = sb.tile([128, RPC], dt)
            nc.sync.dma_start(yitT0[:], yitT_d[0:128, :])
            nc.sync.dma_start(yitT1[:], yitT_d[128:256, :])
            ylcat = sb.tile([RPC, 2 * D], dt)
            nc.sync.dma_start(ylcat[:], ylcat_d[:, :])
            eyeB = sb.tile([RPC, N], dt)
            nc.sync.dma_start(eyeB[:], eyeB_d[:, :])
            i32 = sb.tile([RPC, RPC], dt)
            nc.sync.dma_start(i32[:], i32_d[:, :])
            eyeN = sb.tile([RPC, N], dt)
            nc.sync.dma_start(eyeN[:], eyeN_d[:, :])
            ones = sb.tile([128, RPC], dt)
            nc.vector.memset(ones[:], 1.0)
            cEPS = sb.tile([128, 1], dt)
            nc.vector.memset(cEPS[:], EPS)
            cHALF = sb.tile([128, 1], dt)
            nc.vector.memset(cHALF[:], 0.5)

            # ---- column norms of yi: s_j = sum_d yi[j,d]^2 via ones-matmul
            sq0 = sb.tile([128, N], dt)
            sq1 = sb.tile([128, N], dt)
            nc.vector.tensor_tensor(sq0[:], yiT0[:], yiT0[:], op=Alu.mult)
            nc.vector.tensor_tensor(sq1[:], yiT1[:], yiT1[:], op=Alu.mult)
            ps_s = ps.tile([1, N], dt)
            nc.tensor.matmul(ps_s[:], ones[:, 0:1], sq0[:], start=True, stop=False)
            nc.tensor.matmul(ps_s[:], ones[:, 0:1], sq1[:], start=False, stop=True)
            t_row = sb.tile([1, N], dt)
            nc.scalar.activation(t_row[:], ps_s[:], Act.Sqrt, bias=cEPS[0:1, :], scale=1.0)
            inv_row = sb.tile([1, N], dt)
            nc.vector.reciprocal(inv_row[:], t_row[:])
            # broadcast inv_row down 32 partitions via K=1 matmul
            ps_b = ps.tile([RPC, N], dt)
            nc.tensor.matmul(ps_b[:], ones[0:1, 0:RPC], inv_row[:], start=True, stop=True)

            # ---- raw Gram matrices (local rows x all)
            ps_R = ps.tile([RPC, N], dt)
            nc.tensor.matmul(ps_R[:], yiLT0[:], yiT0[:], start=True, stop=False)
            nc.tensor.matmul(ps_R[:], yiLT1[:], yiT1[:], start=False, stop=False)
            nc.tensor.matmul(ps_R[:], i32[:], eyeN[:], start=False, stop=True)
            ps_Rt = ps.tile([RPC, N], dt)
            nc.tensor.matmul(ps_Rt[:], yitT0[:], yiT0[:], start=True, stop=False)
            nc.tensor.matmul(ps_Rt[:], yitT1[:], yiT1[:], start=False, stop=True)

            # ---- norms of local yi and yi_t rows in one TT+reduce pass
            scrN = sb.tile([RPC, 2 * D], dt)
            nc.vector.tensor_tensor(scrN[:], ylcat[:], ylcat[:], op=Alu.mult)
            nrm2 = sb.tile([RPC, 2], dt)
            nc.vector.tensor_reduce(
                nrm2[:], scrN[:].rearrange("p (g x) -> p g x", g=2),
                axis=mybir.AxisListType.X, op=Alu.add)
            t2 = sb.tile([RPC, 2], dt)
            nc.scalar.activation(t2[:], nrm2[:], Act.Sqrt, bias=cEPS[0:RPC, :], scale=1.0)
            inv2 = sb.tile([RPC, 2], dt)
            nc.vector.reciprocal(inv2[:], t2[:])
            sc_loc = sb.tile([RPC, 1], dt)
            nc.vector.tensor_scalar_mul(sc_loc[:], inv2[:, 0:1], -0.5)
            sc_t = sb.tile([RPC, 1], dt)
            nc.vector.tensor_scalar_mul(sc_t[:], inv2[:, 1:2], -0.5)
            sc_tB = sb.tile([RPC, 1], dt)
            nc.vector.tensor_scalar_mul(sc_tB[:], inv2[:, 1:2], -0.5 / BIG)

            # ---- column-normalized Grams (row scale folded into ACT later)
            # (compiler rejects two PSUM operands in one TensorTensor)
            b_sb = sb.tile([RPC, N], dt)
            nc.vector.tensor_copy(b_sb[:], ps_b[:])
            work = sb.tile([RPC, N], dt)
            nc.vector.tensor_tensor(work[:], ps_R[:], b_sb[:], op=Alu.mult)
            H1 = sb.tile([RPC, N], dt)
            nc.vector.tensor_tensor(H1[:], ps_Rt[:], b_sb[:], op=Alu.mult)

            # dis[i,j] = 0.5*sqrt(2-2*cos) = sqrt(-0.5*inv_i*G1 + 0.5)
            dis = sb.tile([RPC, N], dt)
            nc.scalar.activation(dis[:], work[:], Act.Sqrt, bias=cHALF[0:RPC, :], scale=sc_loc[:])
            dis_t = sb.tile([RPC, N], dt)
            nc.scalar.activation(dis_t[:], H1[:], Act.Sqrt, bias=cHALF[0:RPC, :], scale=sc_t[:])

            # ---- top-16 neighbor threshold per row (self already pushed low)
            m1 = sb.tile([RPC, 8], dt)
            nc.vector.max(out=m1[:], in_=work[:])
            w2 = sb.tile([RPC, N], dt)
            nc.vector.match_replace(
                out=w2[:], in_to_replace=m1[:], in_values=work[:], imm_value=-BIG
            )
            m2 = sb.tile([RPC, 8], dt)
            nc.vector.max(out=m2[:], in_=w2[:])
            mask = sb.tile([RPC, N], dt)
            nc.vector.tensor_scalar(
                mask[:], work[:], m2[:, 7:8], None, op0=Alu.is_ge
            )

            # ---- e1 = sum over neighbors of (dis - dis_t)^2
            diff = sb.tile([RPC, N], dt)
            nc.vector.tensor_sub(diff[:], dis[:], dis_t[:])
            mdiff = sb.tile([RPC, N], dt)
            nc.vector.tensor_tensor(mdiff[:], diff[:], mask[:], op=Alu.mult)
            scrC = sb.tile([RPC, N], dt)
            nc.vector.tensor_tensor(scrC[:], mdiff[:], mdiff[:], op=Alu.mult)
            e1row = sb.tile([RPC, 1], dt)
            nc.vector.tensor_reduce(e1row[:], scrC[:], axis=mybir.AxisListType.X, op=Alu.add)

            # ---- e2 = sum relu(dis(yi,yit) + margin - second_nn)
            scrD = sb.tile([RPC, N], dt)
            nc.vector.tensor_tensor(scrD[:], H1[:], eyeB[:], op=Alu.mult)
            hd2 = sb.tile([RPC, 1], dt)
            nc.vector.tensor_reduce(hd2[:], scrD[:], axis=mybir.AxisListType.X, op=Alu.add)
            dis_ii = sb.tile([RPC, 1], dt)
            nc.scalar.activation(dis_ii[:], hd2[:], Act.Sqrt, bias=cHALF[0:RPC, :], scale=sc_tB[:])
            dis2 = sb.tile([RPC, 1], dt)
            nc.scalar.activation(dis2[:], m1[:, 0:1], Act.Sqrt, bias=cHALF[0:RPC, :], scale=sc_loc[:])
            bias2 = sb.tile([RPC, 1], dt)
            nc.vector.tensor_scalar(
                bias2[:], dis2[:], -1.0, M_MARGIN, op0=Alu.mult, op1=Alu.add
            )
            e2row = sb.tile([RPC, 1], dt)
            nc.scalar.activation(e2row[:], dis_ii[:], Act.Relu, bias=bias2[:], scale=1.0)

            # ---- combine + partition-reduce via ones-matmul
            tot = sb.tile([RPC, 1], dt)
            nc.vector.tensor_add(tot[:], e1row[:], e2row[:])
            ps_f = ps.tile([1, 1], dt)
            nc.tensor.matmul(ps_f[:], ones[0:RPC, 0:1], tot[:], start=True, stop=True)
            outsb = sb.tile([1, 1], dt)
            nc.vector.tensor_scalar_add(outsb[:], ps_f[:], -float(RPC * K * T_THRESH))
            nc.sync.dma_start(out_d[:], outsb[:])

    nc.compile()
    return nc


def _in_maps(yi, yi_t):
    yi = np.ascontiguousarray(np.asarray(yi, np.float32))
    yi_t = np.ascontiguousarray(np.asarray(yi_t, np.float32))
    yiT = np.ascontiguousarray(yi.T)
    maps = []
    for c in range(NCORES):
        r0 = c * RPC
        eyeB = np.zeros((RPC, N), np.float32)
        eyeB[np.arange(RPC), r0 + np.arange(RPC)] = BIG
        maps.append({
            "yiT": yiT,
            "yiLT": np.ascontiguousarray(yi[r0:r0 + RPC].T),
            "yitT": np.ascontiguousarray(yi_t[r0:r0 + RPC].T),
            "ylcat": np.ascontiguousarray(
                np.hstack([yi[r0:r0 + RPC], yi_t[r0:r0 + RPC]])),
            "eyeB": eyeB,
            "i32": np.eye(RPC, dtype=np.float32),
            "eyeN": -eyeB,
        })
    return maps


def kernel(yi, yi_t):
    from concourse.bass_utils import run_bass_kernel_spmd

    if "nc" not in _CACHE:
        _CACHE["nc"] = _build()
    nc = _CACHE["nc"]
    res = run_bass_kernel_spmd(nc, _in_maps(yi, yi_t), list(range(NCORES)))
    partials = [res.results[c]["out"][0, 0] for c in range(NCORES)]
    return np.float32(np.sum(partials, dtype=np.float64))



# revision 6
# speedup vs baseline: 1.1702x; 1.1702x over previous
"""Trainium2 Bass kernel for nn_BLCD_Loss (retrieval_knn).

Math: for l2-normalized rows u_i = yi_i/|yi_i|, v_i = yit_i/|yit_i| and
half-distances d = 0.5*sqrt(2-2c), the e1 summand collapses to cosine
space with a single elementwise sqrt:

  (d_ij - dt_ij)^2 = 1 - 0.5*(C_ij + Ct_ij) - 0.5*sqrt((2-2C_ij)(2-2Ct_ij))

Top-16 neighbor selection per row = two rounds of the 8-wide DVE max /
match_replace on the self-excluded cosine matrix; the sum of the top-16
cosines falls out of the m1/m2 value registers; the two nonlinear masked
sums use fused scalar_tensor_tensor (is_ge mask * value + accumulator).

Engine split per core (32 anchor rows x 256 database rows):
  PE:   Gram matmuls in float32r (1 cycle/row), tiny transpose matmuls
  Pool: partition-axis reductions (column norms), broadcast, elementwise
  DVE:  normalize+exclude, topk chain, masked accumulating reductions
  ACT:  rsqrt/sqrt/relu passes, PSUM->SBUF scalar copies

Sharding: 256 anchor rows -> 32 per core on 8 cores; each core receives
one packed [128, 641] f32 DMA (yi^T both halves, local yi_t^T, local
yi^T, and the row-offset scalar); host sums the 8 scalar partials.
"""

import numpy as np

N, D, K = 256, 256, 16
M_MARGIN, T_THRESH, EPS = 0.6, 0.0025, 1e-12
NCORES, RPC = 8, 32
BIG = 1.0e5
PKW = 2 * D + 4 * RPC + 1  # 641

_CACHE = {}


def _build():
    from concourse import bacc, mybir, tile
    import concourse.bass as bass

    dt = mybir.dt.float32
    dtr = mybir.dt.float32r
    Alu = mybir.AluOpType
    Act = mybir.ActivationFunctionType
    AX = mybir.AxisListType

    nc = bacc.Bacc("TRN2", target_bir_lowering=False, debug=False)

    pk_d = nc.dram_tensor("pk", [128, PKW], dt, kind="ExternalInput")
    out_d = nc.dram_tensor("out", [1, 1], dt, kind="ExternalOutput")

    with tile.TileContext(nc) as tc:
        with (
            tc.tile_pool(name="sb", bufs=1) as sb,
            tc.tile_pool(name="ps", bufs=1, space=bass.MemorySpace.PSUM) as ps,
        ):
            # ---------------- constants (overlap the input DMA) ------------
            c1 = sb.tile([1, 1], dt)
            nc.vector.memset(c1[:], 1.0)
            cEPS = sb.tile([1, 1], dt)
            nc.vector.memset(cEPS[:], EPS)
            cEPS32 = sb.tile([RPC, 1], dt)
            nc.vector.memset(cEPS32[:], EPS)
            cHALF = sb.tile([RPC, 1], dt)
            nc.vector.memset(cHALF[:], 0.5)
            cNH = sb.tile([RPC, 1], dt)
            nc.vector.memset(cNH[:], -0.5)
            ones128 = sb.tile([128, 1], dt)
            nc.vector.memset(ones128[:], 1.0)

            # iota row index j - partition p (f32 exact for small ints)
            ii = sb.tile([RPC, N], dt)
            nc.gpsimd.iota(ii[:], pattern=[[1, N]], base=0,
                           channel_multiplier=-1,
                           allow_small_or_imprecise_dtypes=True)

            # PE warmup for the p-state ramp
            ps_w = ps.tile([1, 1], dt)
            nc.tensor.matmul(ps_w[:], c1[:], c1[:], start=True, stop=True)
            nc.tensor.matmul(ps_w[:], c1[:], c1[:], start=True, stop=True)

            # ---------------- single packed input DMA ----------------------
            pk = sb.tile([128, PKW], dt)
            nc.sync.dma_start(pk[:].bitcast(dtr), pk_d[:, :].bitcast(dtr))
            yiT0 = pk[:, 0:D]
            yiT1 = pk[:, D:2 * D]
            ytT0 = pk[:, 512:544]
            ylT0 = pk[:, 544:576]
            ytT1 = pk[:, 576:608]
            ylT1 = pk[:, 608:640]
            iib = pk[0:RPC, 640:641]  # float(r0) on partitions 0..31

            # ---------------- Gram matmuls (float32r, warm PE) --------------
            ps_R = ps.tile([RPC, N], dt)
            nc.tensor.matmul(ps_R[:], ylT0.bitcast(dtr), yiT0.bitcast(dtr),
                             start=True, stop=False)
            nc.tensor.matmul(ps_R[:], ylT1.bitcast(dtr), yiT1.bitcast(dtr),
                             start=False, stop=True)
            ps_Rt = ps.tile([RPC, N], dt)
            nc.tensor.matmul(ps_Rt[:], ytT0.bitcast(dtr), yiT0.bitcast(dtr),
                             start=True, stop=False)
            nc.tensor.matmul(ps_Rt[:], ytT1.bitcast(dtr), yiT1.bitcast(dtr),
                             start=False, stop=True)

            # ---------------- column norms of yi (Pool + DVE + ACT) ---------
            sq1 = sb.tile([128, N], dt)
            nc.vector.tensor_tensor(sq1[:], yiT1, yiT1, op=Alu.mult)  # DVE
            sq0 = sb.tile([128, N], dt)
            nc.gpsimd.tensor_tensor(sq0[:], yiT0, yiT0, op=Alu.mult)  # Pool
            red0 = sb.tile([1, N], dt)
            nc.gpsimd.tensor_reduce(red0[:], sq0[:], axis=AX.C, op=Alu.add)
            red1 = sb.tile([1, N], dt)
            nc.gpsimd.tensor_reduce(red1[:], sq1[:], axis=AX.C, op=Alu.add)
            ssum = sb.tile([1, N], dt)
            nc.gpsimd.tensor_tensor(ssum[:], red0[:], red1[:], op=Alu.add)
            t_row = sb.tile([1, N], dt)
            nc.scalar.activation(t_row[:], ssum[:], Act.Sqrt,
                                 bias=cEPS[:], scale=1.0)
            inv_row = sb.tile([1, N], dt)
            nc.vector.reciprocal_approx_fast(inv_row[:], t_row[:])

            # local column norms of yi_t / yi: (yt|yl) squared, C-reduced
            sqA = sb.tile([128, 2 * RPC], dt)
            nc.gpsimd.tensor_tensor(sqA[:], pk[:, 512:576], pk[:, 512:576],
                                    op=Alu.mult)
            sqB = sb.tile([128, 2 * RPC], dt)
            nc.gpsimd.tensor_tensor(sqB[:], pk[:, 576:640], pk[:, 576:640],
                                    op=Alu.mult)
            sAB = sb.tile([128, 2 * RPC], dt)
            nc.gpsimd.tensor_tensor(sAB[:], sqA[:], sqB[:], op=Alu.add)
            red_t = sb.tile([1, 2 * RPC], dt)
            nc.gpsimd.tensor_reduce(red_t[:], sAB[:], axis=AX.C, op=Alu.add)

            # self-exclusion mask (-BIG at column r0+i of local row i)
            negBigEye = sb.tile([RPC, N], dt)
            nc.gpsimd.tensor_scalar(negBigEye[:], ii[:], iib, -BIG,
                                    op0=Alu.is_equal, op1=Alu.mult)

            # transposes -> per-partition scalars; rsqrt them on ACT
            ps_sc = ps.tile([RPC, 2], dt)
            nc.tensor.matmul(ps_sc[:, 0:1], red_t[0:1, RPC:2 * RPC], c1[:],
                             start=True, stop=True)  # |yi_i|^2
            nc.tensor.matmul(ps_sc[:, 1:2], red_t[0:1, 0:RPC], c1[:],
                             start=True, stop=True)  # |yit_i|^2
            ns = sb.tile([RPC, 2], dt)
            nc.scalar.activation(ns[:], ps_sc[:], Act.Sqrt,
                                 bias=cEPS32[:], scale=1.0)
            invs = sb.tile([RPC, 2], dt)
            nc.vector.reciprocal_approx_fast(invs[:], ns[:])
            inv_i = invs[:, 0:1]
            invt_i = invs[:, 1:2]

            # local raw dots <yi_i, yit_i> on DVE (small) + PE reduce
            qq = sb.tile([128, RPC], dt)
            nc.vector.tensor_tensor(qq[:], ytT0, ylT0, op=Alu.mult)
            qq2 = sb.tile([128, RPC], dt)
            nc.vector.tensor_tensor(qq2[:], ytT1, ylT1, op=Alu.mult)
            qadd = sb.tile([128, RPC], dt)
            nc.vector.tensor_tensor(qadd[:], qq[:], qq2[:], op=Alu.add)
            ps_gtr = ps.tile([1, RPC], dt)
            nc.tensor.matmul(ps_gtr[:], ones128[:], qadd[:], start=True,
                             stop=True)
            gt_row = sb.tile([1, RPC], dt)
            nc.scalar.copy(gt_row[:], ps_gtr[:])
            ps_gt = ps.tile([RPC, 1], dt)
            nc.tensor.matmul(ps_gt[:], gt_row[:], c1[:], start=True, stop=True)


            # broadcast inv_row down partitions, fuse self-exclusion
            bb = sb.tile([RPC, N], dt)
            nc.gpsimd.partition_broadcast(bb[:], inv_row[:])
            b_excl = sb.tile([RPC, N], dt)
            nc.gpsimd.tensor_tensor(b_excl[:], bb[:], negBigEye[:], op=Alu.add)

            # ---------------- cosines + topk (DVE) --------------------------
            W = sb.tile([RPC, N], dt)
            nc.vector.scalar_tensor_tensor(W[:], ps_R[:], inv_i, b_excl[:],
                                           op0=Alu.mult, op1=Alu.mult)
            Wt = sb.tile([RPC, N], dt)
            nc.vector.scalar_tensor_tensor(Wt[:], ps_Rt[:], invt_i, bb[:],
                                           op0=Alu.mult, op1=Alu.mult)

            m1 = sb.tile([RPC, 8], dt)
            nc.vector.max(out=m1[:], in_=W[:])
            w2 = sb.tile([RPC, N], dt)
            nc.vector.match_replace(out=w2[:], in_to_replace=m1[:],
                                    in_values=W[:], imm_value=-BIG)
            m2 = sb.tile([RPC, 8], dt)
            nc.vector.max(out=m2[:], in_=w2[:])
            thr = m2[:, 7:8]

            # U = 2-2C, Ut = 2-2Ct on Pool; sqrt(U*Ut) on ACT
            U = sb.tile([RPC, N], dt)
            nc.gpsimd.tensor_scalar(U[:], W[:], -2.0, 2.0, op0=Alu.mult,
                                    op1=Alu.add)
            Ut = sb.tile([RPC, N], dt)
            nc.gpsimd.tensor_scalar(Ut[:], Wt[:], -2.0, 2.0, op0=Alu.mult,
                                    op1=Alu.add)
            UU = sb.tile([RPC, N], dt)
            nc.gpsimd.tensor_tensor(UU[:], U[:], Ut[:], op=Alu.mult)
            sqU = sb.tile([RPC, N], dt)
            nc.scalar.activation(sqU[:], UU[:], Act.Sqrt, bias=cEPS32[:],
                                 scale=1.0)

            # top-16 cosine sums from m1/m2 (cheap DVE free-axis reduces)
            r1 = sb.tile([RPC, 1], dt)
            nc.vector.tensor_reduce(r1[:], m1[:], axis=AX.X, op=Alu.add)
            r2 = sb.tile([RPC, 1], dt)
            nc.vector.tensor_reduce(r2[:], m2[:], axis=AX.X, op=Alu.add)
            ct = sb.tile([RPC, 1], dt)
            nc.vector.scalar_tensor_tensor(ct[:], ps_gt[:], inv_i, invt_i,
                                           op0=Alu.mult, op1=Alu.mult)

            # masked accumulations (the only ops that need the mask)
            s1o = sb.tile([RPC, N], dt)
            sqp = sb.tile([RPC, 1], dt)
            nc.vector.scalar_tensor_tensor(s1o[:], W[:], thr, sqU[:],
                                           op0=Alu.is_ge, op1=Alu.mult,
                                           accum_out=sqp[:])
            s2o = sb.tile([RPC, N], dt)
            sWt = sb.tile([RPC, 1], dt)
            nc.vector.scalar_tensor_tensor(s2o[:], W[:], thr, Wt[:],
                                           op0=Alu.is_ge, op1=Alu.mult,
                                           accum_out=sWt[:])

            # ---------------- e2 (ACT + Pool tiny ops) ----------------------
            dis_vu = sb.tile([RPC, 1], dt)
            nc.scalar.activation(dis_vu[:], ct[:], Act.Sqrt, bias=cHALF[:],
                                 scale=cNH[:])
            snn = sb.tile([RPC, 1], dt)
            nc.scalar.activation(snn[:], m1[:, 0:1], Act.Sqrt, bias=cHALF[:],
                                 scale=cNH[:])
            nb = sb.tile([RPC, 1], dt)
            nc.gpsimd.tensor_scalar(nb[:], snn[:], -1.0, M_MARGIN,
                                    op0=Alu.mult, op1=Alu.add)
            e2row = sb.tile([RPC, 1], dt)
            nc.scalar.activation(e2row[:], dis_vu[:], Act.Relu, bias=nb[:],
                                 scale=1.0)

            # ---------------- e1 assembly + total (tiny DVE/Pool) -----------
            sC = sb.tile([RPC, 1], dt)
            nc.gpsimd.tensor_tensor(sC[:], r1[:], r2[:], op=Alu.add)
            zS = sb.tile([RPC, 1], dt)
            nc.vector.tensor_tensor(zS[:], sWt[:], sqp[:], op=Alu.add)
            zT = sb.tile([RPC, 1], dt)
            nc.vector.tensor_tensor(zT[:], zS[:], sC[:], op=Alu.add)
            zC = sb.tile([RPC, 1], dt)
            nc.vector.tensor_scalar(zC[:], zT[:], -0.5,
                                    float(K) * (1.0 - T_THRESH),
                                    op0=Alu.mult, op1=Alu.add)
            tot = sb.tile([RPC, 1], dt)
            nc.vector.tensor_tensor(tot[:], zC[:], e2row[:], op=Alu.add)
            fin = sb.tile([1, 1], dt)
            nc.gpsimd.tensor_reduce(fin[:], tot[:], axis=AX.C, op=Alu.add)
            nc.sync.dma_start(out_d[:], fin[:])

    nc.compile()
    return nc


def _in_maps(yi, yi_t):
    yi = np.ascontiguousarray(np.asarray(yi, np.float32))
    yi_t = np.ascontiguousarray(np.asarray(yi_t, np.float32))
    yiT = yi.T  # [D, N]
    maps = []
    for c in range(NCORES):
        r0 = c * RPC
        pk = np.zeros((128, PKW), np.float32)
        pk[:, 0:D] = yiT[0:128]
        pk[:, D:2 * D] = yiT[128:256]
        ytT = yi_t[r0:r0 + RPC].T  # [D, RPC]
        ylT = yi[r0:r0 + RPC].T
        pk[:, 512:544] = ytT[0:128]
        pk[:, 544:576] = ylT[0:128]
        pk[:, 576:608] = ytT[128:256]
        pk[:, 608:640] = ylT[128:256]
        pk[0:RPC, 640] = float(r0)
        maps.append({"pk": pk})
    return maps


def kernel(yi, yi_t):
    from concourse.bass_utils import run_bass_kernel_spmd

    if "nc" not in _CACHE:
        _CACHE["nc"] = _build()
    nc = _CACHE["nc"]
    res = run_bass_kernel_spmd(nc, _in_maps(yi, yi_t), list(range(NCORES)))
    partials = [res.results[c]["out"][0, 0] for c in range(NCORES)]
    return np.float32(np.sum(partials, dtype=np.float64))


# revision 9
# speedup vs baseline: 1.2827x; 1.0962x over previous
"""Trainium2 Bass kernel for nn_BLCD_Loss (retrieval_knn).

Math: for l2-normalized rows u_i = yi_i/|yi_i|, v_i = yit_i/|yit_i| and
half-distances d = 0.5*sqrt(2-2c), the e1 summand collapses to cosine
space with a single elementwise sqrt pass:

  (d_ij - dt_ij)^2 = 1 - 0.5*(C_ij + Ct_ij) - 0.5*sqrt((2-2C_ij)(2-2Ct_ij))

Top-16 neighbor selection per row = two rounds of the 8-wide DVE max /
match_replace on the self-excluded cosine matrix; the sum of the top-16
cosines falls out of the m1/m2 value registers; the two nonlinear masked
sums use fused scalar_tensor_tensor (is_ge mask * value + accumulator).

Engine split per core (32 anchor rows x 256 database rows):
  PE:   Gram + column-sum matmuls in float32r (1 cycle/row), transposes
  DVE:  squares, normalize+exclude, topk chain, masked accum reductions
  ACT:  sqrt passes, relu, PSUM->SBUF copies (both act tables preloaded)
  Pool: iota/self-eye, small local reductions, broadcast, final reduce

Sharding: 256 anchor rows -> 32 per core on 8 cores; each core receives
one packed [128, 641] f32 image via two pipelined DMAs; the host sums
the 8 scalar partials.
"""

import numpy as np

N, D, K = 256, 256, 16
M_MARGIN, T_THRESH, EPS = 0.6, 0.0025, 1e-12
NCORES, RPC = 8, 32
BIG = 1.0e5
PKW = 2 * D + 4 * RPC + 2  # 642
SPL = D + 4 * RPC + 2      # 386: end of first DMA chunk

_CACHE = {}


def _build():
    from concourse import bacc, mybir, tile
    import concourse.bass as bass

    dt = mybir.dt.float32
    dtr = mybir.dt.float32r
    Alu = mybir.AluOpType
    Act = mybir.ActivationFunctionType
    AX = mybir.AxisListType

    nc = bacc.Bacc("TRN2", target_bir_lowering=False, debug=False)

    pk_d = nc.dram_tensor("pk", [128, PKW], dt, kind="ExternalInput")
    out_d = nc.dram_tensor("out", [1, 1], dt, kind="ExternalOutput")

    with tile.TileContext(nc) as tc:
        with (
            tc.tile_pool(name="sb", bufs=1) as sb,
            tc.tile_pool(name="ps", bufs=1, space=bass.MemorySpace.PSUM) as ps,
        ):
            # -------- PE warmup ASAP (p-state ramp hits full speed by the
            # time the real matmuls arrive); Pool memset is the earliest
            # engine available.
            cW = sb.tile([1, 1], dt)
            nc.gpsimd.memset(cW[:], 1.0)
            ps_w = ps.tile([1, 1], dt)
            nc.tensor.matmul(ps_w[:], cW[:], cW[:], start=True, stop=True)
            nc.tensor.matmul(ps_w[:], cW[:], cW[:], start=True, stop=True)

            # -------- constants ------------------------------------------
            c1 = sb.tile([1, 1], dt)
            nc.vector.memset(c1[:], 1.0)
            cEPS = sb.tile([1, 1], dt)
            nc.vector.memset(cEPS[:], EPS)
            cEPS32 = sb.tile([RPC, 1], dt)
            nc.vector.memset(cEPS32[:], EPS)
            cHALF = sb.tile([RPC, 1], dt)
            nc.vector.memset(cHALF[:], 0.5)
            cNH = sb.tile([RPC, 1], dt)
            nc.vector.memset(cNH[:], -0.5)
            cZ128 = sb.tile([128, 1], dt)
            nc.vector.memset(cZ128[:], 0.0)
            ones_f = sb.tile([128, 1], dt)
            nc.vector.memset(ones_f[:], 1.0)

            # -------- preload BOTH activation tables early ----------------
            d1 = sb.tile([1, 1], dt)
            nc.scalar.activation(d1[:], c1[:], Act.Sqrt, bias=cEPS[:], scale=1.0)
            d2 = sb.tile([1, 1], dt)
            nc.scalar.copy(d2[:], c1[:])

            # iota: ii[p, j] = j - p (f32 exact for small ints)
            ii = sb.tile([RPC, N], dt)
            nc.gpsimd.iota(ii[:], pattern=[[1, N]], base=0,
                           channel_multiplier=-1,
                           allow_small_or_imprecise_dtypes=True)

            # -------- packed input, two pipelined DMAs --------------------
            pk = sb.tile([128, PKW], dt)
            nc.sync.dma_start(pk[:, 0:SPL].bitcast(dtr),
                              pk_d[:, 0:SPL].bitcast(dtr))
            nc.sync.dma_start(pk[:, SPL:PKW].bitcast(dtr),
                              pk_d[:, SPL:PKW].bitcast(dtr))
            yiT0 = pk[:, 0:D]
            ytT0 = pk[:, 256:288]
            ylT0 = pk[:, 288:320]
            ytT1 = pk[:, 320:352]
            ylT1 = pk[:, 352:384]
            iib = pk[0:RPC, 384:385]  # float(r0) on partitions 0..31
            ones_r = pk[:, 385:386].bitcast(dtr)  # DMA-produced f32r ones
            yiT1 = pk[:, SPL:SPL + D]

            # -------- Gram matmuls (float32r, warm PE) --------------------
            ps_R = ps.tile([RPC, N], dt)
            ps_Rt = ps.tile([RPC, N], dt)
            nc.tensor.matmul(ps_R[:], ylT0.bitcast(dtr), yiT0.bitcast(dtr),
                             start=True, stop=False)
            nc.tensor.matmul(ps_Rt[:], ytT0.bitcast(dtr), yiT0.bitcast(dtr),
                             start=True, stop=False)

            # -------- squares on DVE + ACT, column sums on PE -------------
            sq0 = sb.tile([128, N], dtr)
            nc.vector.tensor_tensor(sq0[:], yiT0, yiT0, op=Alu.mult)
            sq1 = sb.tile([128, N], dtr)
            nc.scalar.activation(sq1[:], yiT1, Act.Square,
                                 bias=cZ128[:], scale=1.0)
            ps_s = ps.tile([1, N], dt)
            nc.tensor.matmul(ps_s[:], ones_r, sq0[:], start=True, stop=False)
            nc.tensor.matmul(ps_s[:], ones_r, sq1[:], start=False, stop=True)

            nc.tensor.matmul(ps_R[:], ylT1.bitcast(dtr), yiT1.bitcast(dtr),
                             start=False, stop=True)
            nc.tensor.matmul(ps_Rt[:], ytT1.bitcast(dtr), yiT1.bitcast(dtr),
                             start=False, stop=True)

            # -------- self-exclusion eye + local norm pipeline (Pool) ------
            negBigEye = sb.tile([RPC, N], dt)
            nc.gpsimd.tensor_scalar(negBigEye[:], ii[:], iib, -BIG,
                                    op0=Alu.is_equal, op1=Alu.mult)
            sqA = sb.tile([128, 2 * RPC], dt)
            nc.gpsimd.tensor_tensor(sqA[:], pk[:, 256:320], pk[:, 256:320],
                                    op=Alu.mult)
            sqB = sb.tile([128, 2 * RPC], dt)
            nc.gpsimd.tensor_tensor(sqB[:], pk[:, 320:384], pk[:, 320:384],
                                    op=Alu.mult)
            sAB = sb.tile([128, 2 * RPC], dt)
            nc.gpsimd.tensor_tensor(sAB[:], sqA[:], sqB[:], op=Alu.add)
            red_t = sb.tile([1, 2 * RPC], dt)
            nc.gpsimd.tensor_reduce(red_t[:], sAB[:], axis=AX.C, op=Alu.add)
            qq = sb.tile([128, RPC], dt)
            nc.gpsimd.tensor_tensor(qq[:], ytT0, ylT0, op=Alu.mult)
            qq2 = sb.tile([128, RPC], dt)
            nc.gpsimd.tensor_tensor(qq2[:], ytT1, ylT1, op=Alu.mult)
            qadd = sb.tile([128, RPC], dt)
            nc.gpsimd.tensor_tensor(qadd[:], qq[:], qq2[:], op=Alu.add)

            # transposes -> per-partition squared norms; sqrt on ACT
            ps_sc = ps.tile([RPC, 2], dt)
            nc.tensor.matmul(ps_sc[:, 0:1], red_t[0:1, RPC:2 * RPC], c1[:],
                             start=True, stop=True)  # |yi_i|^2
            nc.tensor.matmul(ps_sc[:, 1:2], red_t[0:1, 0:RPC], c1[:],
                             start=True, stop=True)  # |yit_i|^2
            ps_gtr = ps.tile([1, RPC], dt)
            nc.tensor.matmul(ps_gtr[:], ones_f[:], qadd[:].bitcast(dt),
                             start=True, stop=True)

            # column norms -> 1/norm row; broadcast + fuse self-exclusion
            t_row = sb.tile([1, N], dt)
            nc.scalar.activation(t_row[:], ps_s[:], Act.Sqrt,
                                 bias=cEPS[:], scale=1.0)
            inv_row = sb.tile([1, N], dt)
            nc.vector.reciprocal_approx_fast(inv_row[:], t_row[:])
            bb = sb.tile([RPC, N], dt)
            nc.gpsimd.partition_broadcast(bb[:], inv_row[:])
            b_excl = sb.tile([RPC, N], dt)
            nc.vector.tensor_tensor(b_excl[:], bb[:], negBigEye[:], op=Alu.add)

            ns = sb.tile([RPC, 2], dt)
            nc.scalar.activation(ns[:], ps_sc[:], Act.Sqrt,
                                 bias=cEPS32[:], scale=1.0)
            invs = sb.tile([RPC, 2], dt)
            nc.vector.reciprocal_approx_fast(invs[:], ns[:])
            inv_i = invs[:, 0:1]
            invt_i = invs[:, 1:2]

            gt_row = sb.tile([1, RPC], dt)
            nc.scalar.copy(gt_row[:], ps_gtr[:])
            ps_gt = ps.tile([RPC, 1], dt)
            nc.tensor.matmul(ps_gt[:], gt_row[:], c1[:], start=True, stop=True)

            # -------- cosines + topk (DVE) --------------------------------
            W = sb.tile([RPC, N], dt)
            nc.vector.scalar_tensor_tensor(W[:], ps_R[:], inv_i, b_excl[:],
                                           op0=Alu.mult, op1=Alu.mult)
            Wt = sb.tile([RPC, N], dt)
            nc.vector.scalar_tensor_tensor(Wt[:], ps_Rt[:], invt_i, bb[:],
                                           op0=Alu.mult, op1=Alu.mult)
            U = sb.tile([RPC, N], dt)
            nc.vector.tensor_scalar(U[:], W[:], -2.0, 2.0, op0=Alu.mult,
                                    op1=Alu.add)
            Ut = sb.tile([RPC, N], dt)
            nc.vector.tensor_scalar(Ut[:], Wt[:], -2.0, 2.0, op0=Alu.mult,
                                    op1=Alu.add)
            m1 = sb.tile([RPC, 8], dt)
            nc.vector.max(out=m1[:], in_=W[:])
            w2 = sb.tile([RPC, N], dt)
            nc.vector.match_replace(out=w2[:], in_to_replace=m1[:],
                                    in_values=W[:], imm_value=-BIG)
            m2 = sb.tile([RPC, 8], dt)
            nc.vector.max(out=m2[:], in_=w2[:])
            thr = m2[:, 7:8]

            # sqrt((2-2C)(2-2Ct)) on Pool+ACT, overlapping the topk chain
            UU = sb.tile([RPC, N], dt)
            nc.gpsimd.tensor_tensor(UU[:], U[:], Ut[:], op=Alu.mult)
            sqU = sb.tile([RPC, N], dt)
            nc.scalar.activation(sqU[:], UU[:], Act.Sqrt, bias=cEPS32[:],
                                 scale=1.0)

            # top-16 cosine sums + local cosine (cheap DVE ops)
            r1 = sb.tile([RPC, 1], dt)
            nc.vector.tensor_reduce(r1[:], m1[:], axis=AX.X, op=Alu.add)
            r2 = sb.tile([RPC, 1], dt)
            nc.vector.tensor_reduce(r2[:], m2[:], axis=AX.X, op=Alu.add)
            ct = sb.tile([RPC, 1], dt)
            nc.vector.scalar_tensor_tensor(ct[:], ps_gt[:], inv_i, invt_i,
                                           op0=Alu.mult, op1=Alu.mult)

            # masked accumulations
            s1o = sb.tile([RPC, N], dt)
            sqp = sb.tile([RPC, 1], dt)
            nc.vector.scalar_tensor_tensor(s1o[:], W[:], thr, sqU[:],
                                           op0=Alu.is_ge, op1=Alu.mult,
                                           accum_out=sqp[:])
            s2o = sb.tile([RPC, N], dt)
            sWt = sb.tile([RPC, 1], dt)
            nc.vector.scalar_tensor_tensor(s2o[:], W[:], thr, Wt[:],
                                           op0=Alu.is_ge, op1=Alu.mult,
                                           accum_out=sWt[:])

            # -------- e2 (ACT + Pool tiny ops) ----------------------------
            snn = sb.tile([RPC, 1], dt)
            nc.scalar.activation(snn[:], m1[:, 0:1], Act.Sqrt, bias=cHALF[:],
                                 scale=cNH[:])
            nb = sb.tile([RPC, 1], dt)
            nc.gpsimd.tensor_scalar(nb[:], snn[:], -1.0, M_MARGIN,
                                    op0=Alu.mult, op1=Alu.add)
            dis_vu = sb.tile([RPC, 1], dt)
            nc.scalar.activation(dis_vu[:], ct[:], Act.Sqrt, bias=cHALF[:],
                                 scale=cNH[:])
            e2row = sb.tile([RPC, 1], dt)
            nc.scalar.activation(e2row[:], dis_vu[:], Act.Relu, bias=nb[:],
                                 scale=1.0)

            # -------- e1 assembly + total (tiny DVE ops) ------------------
            sC = sb.tile([RPC, 1], dt)
            nc.gpsimd.tensor_tensor(sC[:], r1[:], r2[:], op=Alu.add)
            zS = sb.tile([RPC, 1], dt)
            nc.vector.tensor_tensor(zS[:], sWt[:], sqp[:], op=Alu.add)
            zT = sb.tile([RPC, 1], dt)
            nc.vector.tensor_tensor(zT[:], zS[:], sC[:], op=Alu.add)
            zC = sb.tile([RPC, 1], dt)
            nc.vector.tensor_scalar(zC[:], zT[:], -0.5,
                                    float(K) * (1.0 - T_THRESH),
                                    op0=Alu.mult, op1=Alu.add)
            tot = sb.tile([RPC, 1], dt)
            nc.vector.tensor_tensor(tot[:], zC[:], e2row[:], op=Alu.add)
            fin = sb.tile([1, 1], dt)
            nc.gpsimd.tensor_reduce(fin[:], tot[:], axis=AX.C, op=Alu.add)
            nc.sync.dma_start(out_d[:], fin[:])

    nc.compile()
    return nc


def _in_maps(yi, yi_t):
    yi = np.ascontiguousarray(np.asarray(yi, np.float32))
    yi_t = np.ascontiguousarray(np.asarray(yi_t, np.float32))
    yiT = yi.T  # [D, N]
    maps = []
    for c in range(NCORES):
        r0 = c * RPC
        pk = np.zeros((128, PKW), np.float32)
        pk[:, 0:D] = yiT[0:128]
        ytT = yi_t[r0:r0 + RPC].T  # [D, RPC]
        ylT = yi[r0:r0 + RPC].T
        pk[:, 256:288] = ytT[0:128]
        pk[:, 288:320] = ylT[0:128]
        pk[:, 320:352] = ytT[128:256]
        pk[:, 352:384] = ylT[128:256]
        pk[0:RPC, 384] = float(r0)
        pk[:, 385] = 1.0
        pk[:, SPL:SPL + D] = yiT[128:256]
        maps.append({"pk": pk})
    return maps


def kernel(yi, yi_t):
    from concourse.bass_utils import run_bass_kernel_spmd

    if "nc" not in _CACHE:
        _CACHE["nc"] = _build()
    nc = _CACHE["nc"]
    res = run_bass_kernel_spmd(nc, _in_maps(yi, yi_t), list(range(NCORES)))
    partials = [res.results[c]["out"][0, 0] for c in range(NCORES)]
    return np.float32(np.sum(partials, dtype=np.float64))


# revision 10
# speedup vs baseline: 1.2899x; 1.0056x over previous
"""Trainium2 Bass kernel for nn_BLCD_Loss (retrieval_knn).

Math: for l2-normalized rows u_i = yi_i/|yi_i|, v_i = yit_i/|yit_i| and
half-distances d = 0.5*sqrt(2-2c), the e1 summand collapses to cosine
space with a single elementwise sqrt pass:

  (d_ij - dt_ij)^2 = 1 - 0.5*(C_ij + Ct_ij) - 0.5*sqrt((2-2C_ij)(2-2Ct_ij))

Top-16 neighbor selection per row = two rounds of the 8-wide DVE max /
match_replace on the self-excluded cosine matrix; the sum of the top-16
cosines falls out of the m1/m2 value registers; the two nonlinear masked
sums use fused scalar_tensor_tensor (is_ge mask * value + accumulator).

Engine split per core (32 anchor rows x 256 database rows):
  PE:   Gram + column-sum matmuls in float32r (1 cycle/row), transposes
  DVE:  squares, normalize+exclude, topk chain, masked accum reductions
  ACT:  sqrt passes, relu, PSUM->SBUF copies (both act tables preloaded)
  Pool: iota/self-eye, small local reductions, broadcast, final reduce

Sharding: 256 anchor rows -> 32 per core on 8 cores; each core receives
one packed [128, 641] f32 image via two pipelined DMAs; the host sums
the 8 scalar partials.
"""

import numpy as np

N, D, K = 256, 256, 16
M_MARGIN, T_THRESH, EPS = 0.6, 0.0025, 1e-12
NCORES, RPC = 8, 32
BIG = 1.0e5
PKW = 2 * D + 4 * RPC + 2  # 642
SPL = D + 4 * RPC + 2      # 386: end of first DMA chunk

_CACHE = {}


def _build():
    from concourse import bacc, mybir, tile
    import concourse.bass as bass

    dt = mybir.dt.float32
    dtr = mybir.dt.float32r
    Alu = mybir.AluOpType
    Act = mybir.ActivationFunctionType
    AX = mybir.AxisListType

    nc = bacc.Bacc("TRN2", target_bir_lowering=False, debug=False)

    pk_d = nc.dram_tensor("pk", [128, PKW], dt, kind="ExternalInput")
    out_d = nc.dram_tensor("out", [RPC, 1], dt, kind="ExternalOutput")

    with tile.TileContext(nc) as tc:
        with (
            tc.tile_pool(name="sb", bufs=1) as sb,
            tc.tile_pool(name="ps", bufs=1, space=bass.MemorySpace.PSUM) as ps,
        ):
            # -------- PE warmup ASAP (p-state ramp hits full speed by the
            # time the real matmuls arrive); Pool memset is the earliest
            # engine available.
            cW = sb.tile([1, 1], dt)
            nc.gpsimd.memset(cW[:], 1.0)
            ps_w = ps.tile([1, 1], dt)
            nc.tensor.matmul(ps_w[:], cW[:], cW[:], start=True, stop=True)
            nc.tensor.matmul(ps_w[:], cW[:], cW[:], start=True, stop=True)

            # -------- constants ------------------------------------------
            c1 = sb.tile([1, 1], dt)
            nc.vector.memset(c1[:], 1.0)
            cEPS = sb.tile([1, 1], dt)
            nc.vector.memset(cEPS[:], EPS)
            cEPS32 = sb.tile([RPC, 1], dt)
            nc.vector.memset(cEPS32[:], EPS)
            cHALF = sb.tile([RPC, 1], dt)
            nc.vector.memset(cHALF[:], 0.5)
            cNH = sb.tile([RPC, 1], dt)
            nc.vector.memset(cNH[:], -0.5)
            cZ128 = sb.tile([128, 1], dt)
            nc.vector.memset(cZ128[:], 0.0)
            ones_f = sb.tile([128, 1], dt)
            nc.vector.memset(ones_f[:], 1.0)

            # -------- preload BOTH activation tables early ----------------
            d1 = sb.tile([1, 1], dt)
            nc.scalar.activation(d1[:], c1[:], Act.Sqrt, bias=cEPS[:], scale=1.0)
            d2 = sb.tile([1, 1], dt)
            nc.scalar.copy(d2[:], c1[:])

            # iota: ii[p, j] = j - p (f32 exact for small ints)
            ii = sb.tile([RPC, N], dt)
            nc.gpsimd.iota(ii[:], pattern=[[1, N]], base=0,
                           channel_multiplier=-1,
                           allow_small_or_imprecise_dtypes=True)

            # -------- packed input, two pipelined DMAs --------------------
            pk = sb.tile([128, PKW], dt)
            nc.sync.dma_start(pk[:, 0:SPL].bitcast(dtr),
                              pk_d[:, 0:SPL].bitcast(dtr))
            nc.sync.dma_start(pk[:, SPL:PKW].bitcast(dtr),
                              pk_d[:, SPL:PKW].bitcast(dtr))
            yiT0 = pk[:, 0:D]
            ytT0 = pk[:, 256:288]
            ylT0 = pk[:, 288:320]
            ytT1 = pk[:, 320:352]
            ylT1 = pk[:, 352:384]
            iib = pk[0:RPC, 384:385]  # float(r0) on partitions 0..31
            ones_r = pk[:, 385:386].bitcast(dtr)  # DMA-produced f32r ones
            yiT1 = pk[:, SPL:SPL + D]

            # -------- Gram matmuls (float32r, warm PE) --------------------
            ps_R = ps.tile([RPC, N], dt)
            ps_Rt = ps.tile([RPC, N], dt)
            nc.tensor.matmul(ps_R[:], ylT0.bitcast(dtr), yiT0.bitcast(dtr),
                             start=True, stop=False)
            nc.tensor.matmul(ps_Rt[:], ytT0.bitcast(dtr), yiT0.bitcast(dtr),
                             start=True, stop=False)

            # -------- squares on DVE + ACT, column sums on PE -------------
            sq0 = sb.tile([128, N], dtr)
            nc.vector.tensor_tensor(sq0[:], yiT0, yiT0, op=Alu.mult)
            sq1 = sb.tile([128, N], dtr)
            nc.scalar.activation(sq1[:], yiT1, Act.Square,
                                 bias=cZ128[:], scale=1.0)
            ps_s = ps.tile([1, N], dt)
            nc.tensor.matmul(ps_s[:], ones_r, sq0[:], start=True, stop=False)
            nc.tensor.matmul(ps_s[:], ones_r, sq1[:], start=False, stop=True)
            nc.tensor.matmul(ps_R[:], ylT1.bitcast(dtr), yiT1.bitcast(dtr),
                             start=False, stop=True)
            nc.tensor.matmul(ps_Rt[:], ytT1.bitcast(dtr), yiT1.bitcast(dtr),
                             start=False, stop=True)
            t_row = sb.tile([1, N], dt)
            nc.scalar.activation(t_row[:], ps_s[:], Act.Sqrt,
                                 bias=cEPS[:], scale=1.0)
            inv_row = sb.tile([1, N], dt)
            nc.vector.reciprocal_approx_fast(inv_row[:], t_row[:])
            bb = sb.tile([RPC, N], dt)
            nc.gpsimd.partition_broadcast(bb[:], inv_row[:])

            # -------- self-exclusion eye + local norm pipeline (Pool) ------
            negBigEye = sb.tile([RPC, N], dt)
            nc.gpsimd.tensor_scalar(negBigEye[:], ii[:], iib, -BIG,
                                    op0=Alu.is_equal, op1=Alu.mult)
            sqA = sb.tile([128, 2 * RPC], dt)
            nc.gpsimd.tensor_tensor(sqA[:], pk[:, 256:320], pk[:, 256:320],
                                    op=Alu.mult)
            sqB = sb.tile([128, 2 * RPC], dt)
            nc.gpsimd.tensor_tensor(sqB[:], pk[:, 320:384], pk[:, 320:384],
                                    op=Alu.mult)
            sAB = sb.tile([128, 2 * RPC], dt)
            nc.gpsimd.tensor_tensor(sAB[:], sqA[:], sqB[:], op=Alu.add)
            red_t = sb.tile([1, 2 * RPC], dt)
            nc.gpsimd.tensor_reduce(red_t[:], sAB[:], axis=AX.C, op=Alu.add)
            qq = sb.tile([128, RPC], dt)
            nc.gpsimd.tensor_tensor(qq[:], ytT0, ylT0, op=Alu.mult)
            qq2 = sb.tile([128, RPC], dt)
            nc.gpsimd.tensor_tensor(qq2[:], ytT1, ylT1, op=Alu.mult)
            qadd = sb.tile([128, RPC], dt)
            nc.gpsimd.tensor_tensor(qadd[:], qq[:], qq2[:], op=Alu.add)

            # transposes -> per-partition squared norms; sqrt on ACT
            ps_sc = ps.tile([RPC, 2], dt)
            nc.tensor.matmul(ps_sc[:, 0:1], red_t[0:1, RPC:2 * RPC], c1[:],
                             start=True, stop=True)  # |yi_i|^2
            nc.tensor.matmul(ps_sc[:, 1:2], red_t[0:1, 0:RPC], c1[:],
                             start=True, stop=True)  # |yit_i|^2
            ps_gtr = ps.tile([1, RPC], dt)
            nc.tensor.matmul(ps_gtr[:], ones_f[:], qadd[:].bitcast(dt),
                             start=True, stop=True)

            # fuse self-exclusion into the broadcast reciprocal row
            b_excl = sb.tile([RPC, N], dt)
            nc.vector.tensor_tensor(b_excl[:], bb[:], negBigEye[:], op=Alu.add)

            ns = sb.tile([RPC, 2], dt)
            nc.scalar.activation(ns[:], ps_sc[:], Act.Sqrt,
                                 bias=cEPS32[:], scale=1.0)
            invs = sb.tile([RPC, 2], dt)
            nc.vector.reciprocal_approx_fast(invs[:], ns[:])
            inv_i = invs[:, 0:1]
            invt_i = invs[:, 1:2]

            gt_row = sb.tile([1, RPC], dt)
            nc.scalar.copy(gt_row[:], ps_gtr[:])
            ps_gt = ps.tile([RPC, 1], dt)
            nc.tensor.matmul(ps_gt[:], gt_row[:], c1[:], start=True, stop=True)

            # -------- cosines + topk (DVE) --------------------------------
            W = sb.tile([RPC, N], dt)
            nc.vector.scalar_tensor_tensor(W[:], ps_R[:], inv_i, b_excl[:],
                                           op0=Alu.mult, op1=Alu.mult)
            Wt = sb.tile([RPC, N], dt)
            nc.vector.scalar_tensor_tensor(Wt[:], ps_Rt[:], invt_i, bb[:],
                                           op0=Alu.mult, op1=Alu.mult)
            U = sb.tile([RPC, N], dt)
            nc.vector.tensor_scalar(U[:], W[:], -2.0, 2.0, op0=Alu.mult,
                                    op1=Alu.add)
            Ut = sb.tile([RPC, N], dt)
            nc.vector.tensor_scalar(Ut[:], Wt[:], -2.0, 2.0, op0=Alu.mult,
                                    op1=Alu.add)
            m1 = sb.tile([RPC, 8], dt)
            nc.vector.max(out=m1[:], in_=W[:])
            w2 = sb.tile([RPC, N], dt)
            nc.vector.match_replace(out=w2[:], in_to_replace=m1[:],
                                    in_values=W[:], imm_value=-BIG)
            m2 = sb.tile([RPC, 8], dt)
            nc.vector.max(out=m2[:], in_=w2[:])
            thr = m2[:, 7:8]

            # sqrt((2-2C)(2-2Ct)) on Pool+ACT, overlapping the topk chain
            UU = sb.tile([RPC, N], dt)
            nc.gpsimd.tensor_tensor(UU[:], U[:], Ut[:], op=Alu.mult)
            sqU = sb.tile([RPC, N], dt)
            nc.scalar.activation(sqU[:], UU[:], Act.Sqrt, bias=cEPS32[:],
                                 scale=1.0)

            # top-16 cosine sums + local cosine (cheap DVE ops)
            r1 = sb.tile([RPC, 1], dt)
            nc.vector.tensor_reduce(r1[:], m1[:], axis=AX.X, op=Alu.add)
            ct = sb.tile([RPC, 1], dt)
            nc.vector.scalar_tensor_tensor(ct[:], ps_gt[:], inv_i, invt_i,
                                           op0=Alu.mult, op1=Alu.mult)

            # masked accumulations
            s1o = sb.tile([RPC, N], dt)
            sqp = sb.tile([RPC, 1], dt)
            nc.vector.scalar_tensor_tensor(s1o[:], W[:], thr, sqU[:],
                                           op0=Alu.is_ge, op1=Alu.mult,
                                           accum_out=sqp[:])
            s2o = sb.tile([RPC, N], dt)
            sWt = sb.tile([RPC, 1], dt)
            nc.vector.scalar_tensor_tensor(s2o[:], W[:], thr, Wt[:],
                                           op0=Alu.is_ge, op1=Alu.mult,
                                           accum_out=sWt[:])
            r2 = sb.tile([RPC, 1], dt)
            nc.vector.tensor_reduce(r2[:], m2[:], axis=AX.X, op=Alu.add)

            # -------- e2 (ACT + Pool tiny ops) ----------------------------
            snn = sb.tile([RPC, 1], dt)
            nc.scalar.activation(snn[:], m1[:, 0:1], Act.Sqrt, bias=cHALF[:],
                                 scale=cNH[:])
            nb = sb.tile([RPC, 1], dt)
            nc.gpsimd.tensor_scalar(nb[:], snn[:], -1.0, M_MARGIN,
                                    op0=Alu.mult, op1=Alu.add)
            dis_vu = sb.tile([RPC, 1], dt)
            nc.scalar.activation(dis_vu[:], ct[:], Act.Sqrt, bias=cHALF[:],
                                 scale=cNH[:])
            e2row = sb.tile([RPC, 1], dt)
            nc.scalar.activation(e2row[:], dis_vu[:], Act.Relu, bias=nb[:],
                                 scale=1.0)

            # -------- e1 assembly + total (tiny DVE ops) ------------------
            sC = sb.tile([RPC, 1], dt)
            nc.gpsimd.tensor_tensor(sC[:], r1[:], r2[:], op=Alu.add)
            zS = sb.tile([RPC, 1], dt)
            nc.vector.tensor_tensor(zS[:], sWt[:], sqp[:], op=Alu.add)
            zT = sb.tile([RPC, 1], dt)
            nc.vector.tensor_tensor(zT[:], zS[:], sC[:], op=Alu.add)
            zC = sb.tile([RPC, 1], dt)
            nc.vector.tensor_scalar(zC[:], zT[:], -0.5,
                                    float(K) * (1.0 - T_THRESH),
                                    op0=Alu.mult, op1=Alu.add)
            tot = sb.tile([RPC, 1], dt)
            nc.vector.tensor_tensor(tot[:], zC[:], e2row[:], op=Alu.add)
            nc.sync.dma_start(out_d[:], tot[:])

    nc.compile()
    return nc


def _in_maps(yi, yi_t):
    yi = np.ascontiguousarray(np.asarray(yi, np.float32))
    yi_t = np.ascontiguousarray(np.asarray(yi_t, np.float32))
    yiT = yi.T  # [D, N]
    maps = []
    for c in range(NCORES):
        r0 = c * RPC
        pk = np.zeros((128, PKW), np.float32)
        pk[:, 0:D] = yiT[0:128]
        ytT = yi_t[r0:r0 + RPC].T  # [D, RPC]
        ylT = yi[r0:r0 + RPC].T
        pk[:, 256:288] = ytT[0:128]
        pk[:, 288:320] = ylT[0:128]
        pk[:, 320:352] = ytT[128:256]
        pk[:, 352:384] = ylT[128:256]
        pk[0:RPC, 384] = float(r0)
        pk[:, 385] = 1.0
        pk[:, SPL:SPL + D] = yiT[128:256]
        maps.append({"pk": pk})
    return maps


def kernel(yi, yi_t):
    from concourse.bass_utils import run_bass_kernel_spmd

    if "nc" not in _CACHE:
        _CACHE["nc"] = _build()
    nc = _CACHE["nc"]
    res = run_bass_kernel_spmd(nc, _in_maps(yi, yi_t), list(range(NCORES)))
    partials = [res.results[c]["out"].sum(dtype=np.float64) for c in range(NCORES)]
    return np.float32(np.sum(partials, dtype=np.float64))


# revision 11
# speedup vs baseline: 1.3770x; 1.0675x over previous
"""Trainium2 Bass kernel for nn_BLCD_Loss (retrieval_knn).

Math: for l2-normalized rows u_i = yi_i/|yi_i|, v_i = yit_i/|yit_i| and
half-distances d = 0.5*sqrt(2-2c), the e1 summand collapses to cosine
space with a single elementwise sqrt pass:

  (d_ij - dt_ij)^2 = 1 - 0.5*(C_ij + Ct_ij) - 0.5*sqrt((2-2C_ij)(2-2Ct_ij))

Top-16 neighbor selection per row = two rounds of the 8-wide DVE max /
match_replace on the self-excluded cosine matrix; the sum of the top-16
cosines falls out of the m1/m2 value registers; the two nonlinear masked
sums use fused scalar_tensor_tensor (is_ge mask * value + accumulator).

Engine split per core (32 anchor rows x 256 database rows):
  PE:   Gram + column-sum matmuls in float32r (1 cycle/row), transposes
  DVE:  squares, normalize+exclude, topk chain, masked accum reductions
  ACT:  sqrt passes, relu, PSUM->SBUF copies (both act tables preloaded)
  Pool: iota/self-eye, small local reductions, broadcast, final reduce

Sharding: 256 anchor rows -> 32 per core on 8 cores; each core receives
one packed [128, 641] f32 image via two pipelined DMAs; the host sums
the 8 scalar partials.
"""

import numpy as np

N, D, K = 256, 256, 16
M_MARGIN, T_THRESH, EPS = 0.6, 0.0025, 1e-12
NCORES, RPC = 8, 32
BIG = 1.0e5
PKW = 2 * D + 4 * RPC + 2  # 642
SPL = D + 4 * RPC + 2      # 386: end of first DMA chunk

_CACHE = {}


def _build():
    from concourse import bacc, mybir, tile
    import concourse.bass as bass

    dt = mybir.dt.float32
    dtr = mybir.dt.float32r
    Alu = mybir.AluOpType
    Act = mybir.ActivationFunctionType
    AX = mybir.AxisListType

    nc = bacc.Bacc("TRN2", target_bir_lowering=False, debug=False)

    pk_d = nc.dram_tensor("pk", [128, PKW], dt, kind="ExternalInput")
    out_d = nc.dram_tensor("out", [RPC, 1], dt, kind="ExternalOutput")

    with tile.TileContext(nc) as tc:
        with (
            tc.tile_pool(name="sb", bufs=1) as sb,
            tc.tile_pool(name="ps", bufs=1, space=bass.MemorySpace.PSUM) as ps,
        ):
            # -------- PE warmup ASAP (p-state ramp hits full speed by the
            # time the real matmuls arrive); Pool memset is the earliest
            # engine available.
            cW = sb.tile([1, 1], dt)
            nc.gpsimd.memset(cW[:], 1.0)
            ps_w = ps.tile([1, 1], dt)
            nc.tensor.matmul(ps_w[:], cW[:], cW[:], start=True, stop=True)
            nc.tensor.matmul(ps_w[:], cW[:], cW[:], start=True, stop=True)

            # -------- constants ------------------------------------------
            c1 = sb.tile([1, 1], dt)
            nc.vector.memset(c1[:], 1.0)
            cEPS = sb.tile([1, 1], dt)
            nc.vector.memset(cEPS[:], EPS)
            cEPS32 = sb.tile([RPC, 1], dt)
            nc.vector.memset(cEPS32[:], EPS)
            cHALF = sb.tile([RPC, 1], dt)
            nc.vector.memset(cHALF[:], 0.5)
            cNH = sb.tile([RPC, 1], dt)
            nc.vector.memset(cNH[:], -0.5)
            cZ128 = sb.tile([128, 1], dt)
            nc.vector.memset(cZ128[:], 0.0)
            ones_f = sb.tile([128, 1], dt)
            nc.vector.memset(ones_f[:], 1.0)

            # -------- preload BOTH activation tables early ----------------
            d1 = sb.tile([1, 1], dt)
            nc.scalar.activation(d1[:], c1[:], Act.Sqrt, bias=cEPS[:], scale=1.0)
            d2 = sb.tile([1, 1], dt)
            nc.scalar.copy(d2[:], c1[:])

            # iota: ii[p, j] = j - p (f32 exact for small ints)
            ii = sb.tile([RPC, N], dt)
            nc.gpsimd.iota(ii[:], pattern=[[1, N]], base=0,
                           channel_multiplier=-1,
                           allow_small_or_imprecise_dtypes=True)

            # -------- packed input, two pipelined DMAs --------------------
            pk = sb.tile([128, PKW], dt)
            nc.sync.dma_start(pk[:, 0:SPL].bitcast(dtr),
                              pk_d[:, 0:SPL].bitcast(dtr))
            nc.sync.dma_start(pk[:, SPL:PKW].bitcast(dtr),
                              pk_d[:, SPL:PKW].bitcast(dtr))
            yiT0 = pk[:, 0:D]
            ytT0 = pk[:, 256:288]
            ylT0 = pk[:, 288:320]
            ytT1 = pk[:, 320:352]
            ylT1 = pk[:, 352:384]
            iib = pk[0:RPC, 384:385]  # float(r0) on partitions 0..31
            ones_r = pk[:, 385:386].bitcast(dtr)  # DMA-produced f32r ones
            yiT1 = pk[:, SPL:SPL + D]

            # -------- Gram matmuls (float32r, warm PE) --------------------
            ps_R = ps.tile([RPC, N], dt)
            ps_Rt = ps.tile([RPC, N], dt)
            nc.tensor.matmul(ps_R[:], ylT0.bitcast(dtr), yiT0.bitcast(dtr),
                             start=True, stop=False)
            nc.tensor.matmul(ps_Rt[:], ytT0.bitcast(dtr), yiT0.bitcast(dtr),
                             start=True, stop=False)

            # -------- squares on DVE + ACT, column sums on PE -------------
            sq0 = sb.tile([128, N], dtr)
            nc.vector.tensor_tensor(sq0[:], yiT0, yiT0, op=Alu.mult)
            sq1 = sb.tile([128, N], dtr)
            nc.vector.tensor_tensor(sq1[:], yiT1, yiT1, op=Alu.mult)
            ps_s = ps.tile([1, N], dt)
            nc.tensor.matmul(ps_s[:], ones_r, sq0[:], start=True, stop=False)
            nc.tensor.matmul(ps_s[:], ones_r, sq1[:], start=False, stop=True)
            nc.tensor.matmul(ps_R[:], ylT1.bitcast(dtr), yiT1.bitcast(dtr),
                             start=False, stop=True)
            nc.tensor.matmul(ps_Rt[:], ytT1.bitcast(dtr), yiT1.bitcast(dtr),
                             start=False, stop=True)
            t_row = sb.tile([1, N], dt)
            nc.scalar.activation(t_row[:], ps_s[:], Act.Sqrt,
                                 bias=cEPS[:], scale=1.0)
            inv_row = sb.tile([1, N], dt)
            nc.vector.reciprocal_approx_fast(inv_row[:], t_row[:])
            bb = sb.tile([RPC, N], dt)
            nc.gpsimd.partition_broadcast(bb[:], inv_row[:])

            # -------- self-exclusion eye + local norm pipeline (Pool) ------
            negBigEye = sb.tile([RPC, N], dt)
            nc.gpsimd.tensor_scalar(negBigEye[:], ii[:], iib, -BIG,
                                    op0=Alu.is_equal, op1=Alu.mult)
            sqA = sb.tile([128, 2 * RPC], dt)
            nc.gpsimd.tensor_tensor(sqA[:], pk[:, 256:320], pk[:, 256:320],
                                    op=Alu.mult)
            sqB = sb.tile([128, 2 * RPC], dt)
            nc.gpsimd.tensor_tensor(sqB[:], pk[:, 320:384], pk[:, 320:384],
                                    op=Alu.mult)
            sAB = sb.tile([128, 2 * RPC], dt)
            nc.gpsimd.tensor_tensor(sAB[:], sqA[:], sqB[:], op=Alu.add)
            red_t = sb.tile([1, 2 * RPC], dt)
            nc.gpsimd.tensor_reduce(red_t[:], sAB[:], axis=AX.C, op=Alu.add)
            qq = sb.tile([128, RPC], dt)
            nc.gpsimd.tensor_tensor(qq[:], ytT0, ylT0, op=Alu.mult)
            qq2 = sb.tile([128, RPC], dt)
            nc.gpsimd.tensor_tensor(qq2[:], ytT1, ylT1, op=Alu.mult)
            qadd = sb.tile([128, RPC], dt)
            nc.gpsimd.tensor_tensor(qadd[:], qq[:], qq2[:], op=Alu.add)

            # transposes -> per-partition squared norms; sqrt on ACT
            ps_sc = ps.tile([RPC, 2], dt)
            nc.tensor.matmul(ps_sc[:, 0:1], red_t[0:1, RPC:2 * RPC], c1[:],
                             start=True, stop=True)  # |yi_i|^2
            nc.tensor.matmul(ps_sc[:, 1:2], red_t[0:1, 0:RPC], c1[:],
                             start=True, stop=True)  # |yit_i|^2
            ps_gtr = ps.tile([1, RPC], dt)
            nc.tensor.matmul(ps_gtr[:], ones_f[:], qadd[:].bitcast(dt),
                             start=True, stop=True)

            # fuse self-exclusion into the broadcast reciprocal row
            b_excl = sb.tile([RPC, N], dt)
            nc.vector.tensor_tensor(b_excl[:], bb[:], negBigEye[:], op=Alu.add)

            ns = sb.tile([RPC, 2], dt)
            nc.scalar.activation(ns[:], ps_sc[:], Act.Sqrt,
                                 bias=cEPS32[:], scale=1.0)
            invs = sb.tile([RPC, 2], dt)
            nc.vector.reciprocal_approx_fast(invs[:], ns[:])
            inv_i = invs[:, 0:1]
            invt_i = invs[:, 1:2]

            gt_row = sb.tile([1, RPC], dt)
            nc.scalar.copy(gt_row[:], ps_gtr[:])
            ps_gt = ps.tile([RPC, 1], dt)
            nc.tensor.matmul(ps_gt[:], gt_row[:], c1[:], start=True, stop=True)

            # -------- cosines + topk (DVE) --------------------------------
            W = sb.tile([RPC, N], dt)
            nc.vector.scalar_tensor_tensor(W[:], ps_R[:], inv_i, b_excl[:],
                                           op0=Alu.mult, op1=Alu.mult)
            Wt = sb.tile([RPC, N], dt)
            nc.vector.scalar_tensor_tensor(Wt[:], ps_Rt[:], invt_i, bb[:],
                                           op0=Alu.mult, op1=Alu.mult)
            U = sb.tile([RPC, N], dt)
            nc.vector.tensor_scalar(U[:], W[:], -2.0, 2.0, op0=Alu.mult,
                                    op1=Alu.add)
            Ut = sb.tile([RPC, N], dt)
            nc.vector.tensor_scalar(Ut[:], Wt[:], -2.0, 2.0, op0=Alu.mult,
                                    op1=Alu.add)
            m1 = sb.tile([RPC, 8], dt)
            nc.vector.max(out=m1[:], in_=W[:])
            w2 = sb.tile([RPC, N], dt)
            nc.vector.match_replace(out=w2[:], in_to_replace=m1[:],
                                    in_values=W[:], imm_value=-BIG)
            m2 = sb.tile([RPC, 8], dt)
            nc.vector.max(out=m2[:], in_=w2[:])
            thr = m2[:, 7:8]

            # sqrt((2-2C)(2-2Ct)) on Pool+ACT, overlapping the topk chain
            UU = sb.tile([RPC, N], dt)
            nc.gpsimd.tensor_tensor(UU[:], U[:], Ut[:], op=Alu.mult)
            sqU = sb.tile([RPC, N], dt)
            nc.scalar.activation(sqU[:], UU[:], Act.Sqrt, bias=cEPS32[:],
                                 scale=1.0)

            # top-16 cosine sums + local cosine (cheap DVE ops)
            r1 = sb.tile([RPC, 1], dt)
            nc.vector.tensor_reduce(r1[:], m1[:], axis=AX.X, op=Alu.add)
            ct = sb.tile([RPC, 1], dt)
            nc.vector.scalar_tensor_tensor(ct[:], ps_gt[:], inv_i, invt_i,
                                           op0=Alu.mult, op1=Alu.mult)

            # masked accumulations
            s1o = sb.tile([RPC, N], dt)
            sqp = sb.tile([RPC, 1], dt)
            nc.vector.scalar_tensor_tensor(s1o[:], W[:], thr, sqU[:],
                                           op0=Alu.is_ge, op1=Alu.mult,
                                           accum_out=sqp[:])
            s2o = sb.tile([RPC, N], dt)
            sWt = sb.tile([RPC, 1], dt)
            nc.vector.scalar_tensor_tensor(s2o[:], W[:], thr, Wt[:],
                                           op0=Alu.is_ge, op1=Alu.mult,
                                           accum_out=sWt[:])
            r2 = sb.tile([RPC, 1], dt)
            nc.vector.tensor_reduce(r2[:], m2[:], axis=AX.X, op=Alu.add)

            # -------- e2 (ACT + Pool tiny ops) ----------------------------
            snn = sb.tile([RPC, 1], dt)
            nc.scalar.activation(snn[:], m1[:, 0:1], Act.Sqrt, bias=cHALF[:],
                                 scale=cNH[:])
            nb = sb.tile([RPC, 1], dt)
            nc.gpsimd.tensor_scalar(nb[:], snn[:], -1.0, M_MARGIN,
                                    op0=Alu.mult, op1=Alu.add)
            dis_vu = sb.tile([RPC, 1], dt)
            nc.scalar.activation(dis_vu[:], ct[:], Act.Sqrt, bias=cHALF[:],
                                 scale=cNH[:])
            e2row = sb.tile([RPC, 1], dt)
            nc.scalar.activation(e2row[:], dis_vu[:], Act.Relu, bias=nb[:],
                                 scale=1.0)

            # -------- e1 assembly + total (tiny DVE ops) ------------------
            sC = sb.tile([RPC, 1], dt)
            nc.gpsimd.tensor_tensor(sC[:], r1[:], r2[:], op=Alu.add)
            zS = sb.tile([RPC, 1], dt)
            nc.vector.tensor_tensor(zS[:], sWt[:], sqp[:], op=Alu.add)
            zT = sb.tile([RPC, 1], dt)
            nc.vector.tensor_tensor(zT[:], zS[:], sC[:], op=Alu.add)
            zC = sb.tile([RPC, 1], dt)
            nc.vector.tensor_scalar(zC[:], zT[:], -0.5,
                                    float(K) * (1.0 - T_THRESH),
                                    op0=Alu.mult, op1=Alu.add)
            tot = sb.tile([RPC, 1], dt)
            nc.vector.tensor_tensor(tot[:], zC[:], e2row[:], op=Alu.add)
            nc.sync.dma_start(out_d[:], tot[:])

    nc.compile()
    return nc


def _in_maps(yi, yi_t):
    yi = np.ascontiguousarray(np.asarray(yi, np.float32))
    yi_t = np.ascontiguousarray(np.asarray(yi_t, np.float32))
    yiT = yi.T  # [D, N]
    maps = []
    for c in range(NCORES):
        r0 = c * RPC
        pk = np.zeros((128, PKW), np.float32)
        pk[:, 0:D] = yiT[0:128]
        ytT = yi_t[r0:r0 + RPC].T  # [D, RPC]
        ylT = yi[r0:r0 + RPC].T
        pk[:, 256:288] = ytT[0:128]
        pk[:, 288:320] = ylT[0:128]
        pk[:, 320:352] = ytT[128:256]
        pk[:, 352:384] = ylT[128:256]
        pk[0:RPC, 384] = float(r0)
        pk[:, 385] = 1.0
        pk[:, SPL:SPL + D] = yiT[128:256]
        maps.append({"pk": pk})
    return maps


def kernel(yi, yi_t):
    from concourse.bass_utils import run_bass_kernel_spmd

    if "nc" not in _CACHE:
        _CACHE["nc"] = _build()
    nc = _CACHE["nc"]
    res = run_bass_kernel_spmd(nc, _in_maps(yi, yi_t), list(range(NCORES)))
    partials = [res.results[c]["out"].sum(dtype=np.float64) for c in range(NCORES)]
    return np.float32(np.sum(partials, dtype=np.float64))
